# revision 12
# baseline (speedup 1.0000x reference)
"""AWQ W4A16-style quantized linear (nn_AWQLinear) on 8 Trainium2 NeuronCores.

y[m,n] = sum_k x[m,k] * ((wq[n,k]*scales[n,g(k)] + zeros[n,g(k)]) / cs[k]) + bias[n]

Column-parallel over out_features (8 cores, N_shard = 1376/core).

Host (layout-only): qweight transposed to byte-rows [K/2, N]; x transposed
(fp16) with a k-permutation so each 128-row k-tile pair shares one byte-row
block; scales replicated to the per-k-row broadcast pattern (fp16 round —
identical to a device-side cast).

Device, per super-pair sp (2 byte-row blocks = 4 k-tiles), all heavy
elementwise fused to minimize per-op overhead:
  nibcat[:, :2N]  = (qbcat.u16 & 0x0f0f).u8          DVE, 16-bit 2x mode
  nibcat[:, 2N:]  = ((qbcat.u16 >> 4) & 0x0f0f).u8   DVE
  wcat = nibcat * srep(broadcast)  -> f16            one TT (DVE or GPSIMD)
  12 matmuls -> 6 psum accumulators; 0/1-pattern matmuls -> S psum
Tail: zeros+bias folded into one augmented matmul (S16 row 32 = 1, zT row 32
= bias); outputs DMA'd directly from PSUM.
"""
import numpy as np

import concourse.bacc as bacc
import concourse.mybir as mybir
from concourse import tile
from concourse.bass_utils import run_bass_kernel_spmd

IN_F = 4096          # K
OUT_F = 11008        # N
M_TOK = 256          # M
NCORES = 8
NSH = OUT_F // NCORES   # 1376
NPAIR = IN_F // 256     # 16 byte-row blocks of 128 rows (each -> 2 k-tiles)
NSP = NPAIR // 2        # 8 super-pairs
CHUNKS = [(0, 512), (512, 512), (1024, NSH - 1024)]
# which super-pairs run their w-mult on GPSIMD (rest on DVE)
POOL_SP = {1, 4, 6}

F32, F16, U8, U16 = mybir.dt.float32, mybir.dt.float16, mybir.dt.uint8, mybir.dt.uint16


def _build_nc():
    nc = bacc.Bacc("TRN2", target_bir_lowering=False, debug=False,
                   num_devices=NCORES)

    xT_d = nc.dram_tensor("xT", [128, 32 * M_TOK], F16, kind="ExternalInput")
    csT_d = nc.dram_tensor("csT", [128, 32], F32, kind="ExternalInput")
    qwT_d = nc.dram_tensor("qwT", [IN_F // 2, NSH], U8, kind="ExternalInput")
    srep_d = nc.dram_tensor("srep", [NPAIR * 128, NSH], F16, kind="ExternalInput")
    zrT_d = nc.dram_tensor("zerosT", [32, NSH], F32, kind="ExternalInput")
    bias_d = nc.dram_tensor("bias", [1, NSH], F32, kind="ExternalInput")
    gpat_d = nc.dram_tensor("gpat", [128, NPAIR * 32], F16, kind="ExternalInput")
    y_d = nc.dram_tensor("y", [M_TOK, NSH], F32, kind="ExternalOutput")

    A = mybir.AluOpType

    with tile.TileContext(nc) as tc:
        with (
            tc.tile_pool(name="const", bufs=1) as cpool,
            tc.tile_pool(name="x16", bufs=1) as x16pool,
            tc.tile_pool(name="qb", bufs=3) as qbpool,
            tc.tile_pool(name="srep", bufs=3) as sreppool,
            tc.tile_pool(name="nib", bufs=3) as nibpool,
            tc.tile_pool(name="w", bufs=3) as wpool,
            tc.tile_pool(name="yout", bufs=2) as ypool,
            tc.tile_pool(name="ps", bufs=1, space="PSUM") as pspool,
        ):
            # ---- constants / small tensors ----
            csT = cpool.tile([128, 32], F32)
            nc.scalar.dma_start(csT[:], csT_d[:])
            rcs = cpool.tile([128, 32], F32)
            nc.vector.reciprocal(rcs[:], csT[:])

            zrT32 = cpool.tile([32, NSH], F32)
            nc.scalar.dma_start(zrT32[:], zrT_d[:])
            zT16 = cpool.tile([33, NSH], F16)
            nc.scalar.copy(zT16[:32, :], zrT32[:])
            b32 = cpool.tile([1, NSH], F32)
            nc.scalar.dma_start(b32[:], bias_d[:])
            nc.scalar.copy(zT16[32:33, :], b32[:])

            gpat = cpool.tile([128, NPAIR * 32], F16)
            nc.scalar.dma_start(gpat[:], gpat_d[:])

            # ---- x: 8 chunk DMAs (f16) into separate tiles, ACT converts ----
            w4 = 4 * M_TOK
            xraw = []
            for c in range(8):
                xr = x16pool.tile([128, w4], F16, tag=f"xraw_{c}",
                                  name=f"xraw_{c}")
                nc.scalar.dma_start(xr[:], xT_d[:, c * w4:(c + 1) * w4])
                xraw.append(xr)
            x16 = []
            for t in range(32):
                xt = x16pool.tile([128, M_TOK], F16, tag=f"x16_{t}",
                                  name=f"x16_{t}")
                src = xraw[t // 4][:, (t % 4) * M_TOK:(t % 4 + 1) * M_TOK]
                nc.scalar.mul(xt[:], src, rcs[:, t:t + 1])
                x16.append(xt)

            # ---- psum accumulators ----
            y_ps = [[pspool.tile([128, w], F32, tag=f"yps_{m}_{ci}",
                                 name=f"yps_{m}_{ci}")
                     for ci, (_, w) in enumerate(CHUNKS)] for m in range(2)]
            S_ps = pspool.tile([32, M_TOK], F32, tag="S_ps")

            # ---- main loop over super-pairs (2 byte-row blocks each) ----
            for sp in range(NSP):
                # qbcat: [pairA bytes | pairB bytes]
                qbcat = qbpool.tile([128, 2 * NSH], U8, tag="qb",
                                    name=f"qb_{sp}")
                nc.sync.dma_start(
                    qbcat[:].rearrange("p (j n) -> p j n", j=2),
                    qwT_d[sp * 256:(sp + 1) * 256, :]
                    .rearrange("(j p) n -> p j n", p=128))

                srepc = sreppool.tile([128, 2 * NSH], F16, tag="srep",
                                      name=f"srep_{sp}")
                nc.sync.dma_start(
                    srepc[:].rearrange("p (j n) -> p j n", j=2),
                    srep_d[sp * 256:(sp + 1) * 256, :]
                    .rearrange("(j p) n -> p j n", p=128))

                # nibcat: [loA | loB | hiA | hiB]
                nibcat = nibpool.tile([128, 4 * NSH], U8, tag="nib",
                                      name=f"nib_{sp}")
                nc.vector.tensor_scalar(
                    nibcat[:, :2 * NSH].bitcast(U16), in0=qbcat[:].bitcast(U16),
                    scalar1=0x0F0F, scalar2=None, op0=A.bitwise_and)
                nc.vector.tensor_scalar(
                    nibcat[:, 2 * NSH:].bitcast(U16), in0=qbcat[:].bitcast(U16),
                    scalar1=4, scalar2=0x0F0F,
                    op0=A.logical_shift_right, op1=A.bitwise_and)

                # wcat = nibcat * [srepc | srepc] in one TT
                wcat = wpool.tile([128, 4 * NSH], F16, tag="w",
                                  name=f"w_{sp}")
                eng = nc.gpsimd if sp in POOL_SP else nc.vector
                eng.tensor_tensor(
                    wcat[:].rearrange("p (r c) -> p r c", r=2),
                    nibcat[:].rearrange("p (r c) -> p r c", r=2),
                    srepc[:, None, :].to_broadcast((128, 2, 2 * NSH)),
                    A.mult)

                # tiles: wcat layout: [loA loB hiA hiB] -> k-tiles
                # pairA = blocks (lo at 0, hi at 2), pairB = (lo at 1, hi at 3)
                for j in range(2):          # pair within super-pair
                    b = 2 * sp + j
                    # group-sum matmuls
                    nc.tensor.matmul(S_ps[:], gpat[:, b * 32:(b + 1) * 32],
                                     x16[2 * b][:], start=(b == 0), stop=False)
                    nc.tensor.matmul(S_ps[:], gpat[:, b * 32:(b + 1) * 32],
                                     x16[2 * b + 1][:], start=False,
                                     stop=(b == NPAIR - 1))
                    for half, xt in ((0, x16[2 * b]), (1, x16[2 * b + 1])):
                        w0 = (2 * half + j) * NSH
                        for m in range(2):
                            for ci, (c0, cw) in enumerate(CHUNKS):
                                nc.tensor.matmul(
                                    y_ps[m][ci][:],
                                    xt[:, m * 128:(m + 1) * 128],
                                    wcat[:, w0 + c0:w0 + c0 + cw],
                                    start=(b == 0 and half == 0),
                                    stop=False,
                                )

            # ---- tail: zeros+bias augmented matmul, direct-psum stores ----
            S16 = cpool.tile([33, M_TOK], F16)
            nc.scalar.copy(S16[:32, :], S_ps[:])
            nc.vector.memset(S16[32:33, :], 1.0)
            for m in range(2):
                for ci, (c0, cw) in enumerate(CHUNKS):
                    nc.tensor.matmul(y_ps[m][ci][:],
                                     S16[:, m * 128:(m + 1) * 128],
                                     zT16[:, c0:c0 + cw],
                                     start=False, stop=True)
                    ysb = ypool.tile([128, cw], F32, tag=f"ysb_{ci}",
                                     name=f"ysb_{m}_{ci}")
                    nc.scalar.copy(ysb[:], y_ps[m][ci][:])
                    nc.scalar.dma_start(y_d[m * 128:(m + 1) * 128, c0:c0 + cw],
                                        ysb[:])
    nc.compile()
    return nc


def _host_prep(x, qweight, scales, zeros, channel_scales, bias):
    x2 = np.asarray(x, dtype=np.float32).reshape(M_TOK, IN_F)
    qw = np.asarray(qweight)
    if qw.dtype != np.uint8:
        qw = qw.astype(np.uint8)
    qwT = np.ascontiguousarray(qw.T)                      # [K/2, N]

    q = np.arange(128)
    perm = np.empty(IN_F, np.int64)
    for b in range(NPAIR):
        perm[(2 * b) * 128 + q] = 256 * b + 2 * q
        perm[(2 * b + 1) * 128 + q] = 256 * b + 2 * q + 1

    xT_perm = x2.T[perm]                                  # [K, M]
    xT_b = np.ascontiguousarray(
        xT_perm.reshape(32, 128, M_TOK).transpose(1, 0, 2)
        .reshape(128, 32 * M_TOK)).astype(np.float16)
    cs_perm = np.asarray(channel_scales, np.float32)[perm]
    csT = np.ascontiguousarray(cs_perm.reshape(32, 128).T)  # [128, 32]

    scalesT = np.asarray(scales, np.float32).T            # [32, N]
    srep = np.empty((NPAIR * 128, OUT_F), np.float16)
    for b in range(NPAIR):
        srep[b * 128:b * 128 + 64] = scalesT[2 * b].astype(np.float16)
        srep[b * 128 + 64:(b + 1) * 128] = scalesT[2 * b + 1].astype(np.float16)

    zerosT = np.ascontiguousarray(np.asarray(zeros, np.float32).T)
    bias_f = np.asarray(bias, np.float32)

    gpat = np.zeros((128, NPAIR * 32), np.float16)
    for b in range(NPAIR):
        gpat[0:64, b * 32 + 2 * b] = 1.0
        gpat[64:128, b * 32 + 2 * b + 1] = 1.0

    return xT_b, csT, qwT, srep, zerosT, bias_f, gpat


def make_in_maps(x, qweight, scales, zeros, channel_scales, bias):
    xT_b, csT, qwT, srep, zerosT, bias_f, gpat = _host_prep(
        x, qweight, scales, zeros, channel_scales, bias)
    in_maps = []
    for c in range(NCORES):
        sl = slice(c * NSH, (c + 1) * NSH)
        in_maps.append({
            "xT": xT_b,
            "csT": csT,
            "qwT": np.ascontiguousarray(qwT[:, sl]),
            "srep": np.ascontiguousarray(srep[:, sl]),
            "zerosT": np.ascontiguousarray(zerosT[:, sl]),
            "bias": np.ascontiguousarray(bias_f[sl]).reshape(1, NSH),
            "gpat": gpat,
        })
    return in_maps


_NC_CACHE = {}


def get_nc():
    if "nc" not in _NC_CACHE:
        _NC_CACHE["nc"] = _build_nc()
    return _NC_CACHE["nc"]


def kernel(x, qweight, scales, zeros, channel_scales, bias):
    in_maps = make_in_maps(x, qweight, scales, zeros, channel_scales, bias)
    nc = get_nc()
    res = run_bass_kernel_spmd(nc, in_maps, core_ids=list(range(NCORES)))
    y = np.concatenate([res.results[c]["y"] for c in range(NCORES)], axis=1)
    return y.reshape(1, M_TOK, OUT_F).astype(np.float32)


# revision 15
# speedup vs baseline: 1.2444x; 1.2444x over previous
"""AWQ W4A16-style quantized linear (nn_AWQLinear) on 8 Trainium2 NeuronCores.

y[m,n] = sum_k x[m,k] * ((wq[n,k]*scales[n,g(k)] + zeros[n,g(k)]) / cs[k]) + bias[n]

Column-parallel over out_features (8 cores, N_shard = 1376/core).

Host (layout-only): qweight transposed to byte-rows [K/2, N]; x transposed
(fp16) with a k-permutation so each 128-row k-tile pair shares one byte-row
block; scales replicated to the per-k-row broadcast pattern (fp16 round —
identical to a device-side cast).

Device, per super-pair sp (2 byte-row blocks = 4 k-tiles), all heavy
elementwise fused to minimize per-op overhead:
  nibcat[:, :2N]  = (qbcat.u16 & 0x0f0f).u8          DVE, 16-bit 2x mode
  nibcat[:, 2N:]  = ((qbcat.u16 >> 4) & 0x0f0f).u8   DVE
  wcat = nibcat * srep(broadcast)  -> f16            one TT (DVE or GPSIMD)
  12 matmuls -> 6 psum accumulators; 0/1-pattern matmuls -> S psum
Tail: zeros+bias folded into one augmented matmul (S16 row 32 = 1, zT row 32
= bias); outputs DMA'd directly from PSUM.
"""
import numpy as np

import concourse.bacc as bacc
import concourse.mybir as mybir
from concourse import tile
from concourse.bass_utils import run_bass_kernel_spmd

IN_F = 4096          # K
OUT_F = 11008        # N
M_TOK = 256          # M
NCORES = 8
NSH = OUT_F // NCORES   # 1376
NPAIR = IN_F // 256     # 16 byte-row blocks of 128 rows (each -> 2 k-tiles)
NSP = NPAIR // 2        # 8 super-pairs
CHUNKS = [(0, 512), (512, 512), (1024, NSH - 1024)]
# (pair, half) keys whose w-mult runs on GPSIMD (rest on DVE)
POOL_TT = {(b, h) for b in range(NPAIR) for h in range(2)
           if (2 * b + h) % 3 == 1}
LOOKAHEAD = 2  # super-pairs of dequant emitted ahead of their matmuls

F32, F16, U8, U16 = mybir.dt.float32, mybir.dt.float16, mybir.dt.uint8, mybir.dt.uint16


def _build_nc():
    nc = bacc.Bacc("TRN2", target_bir_lowering=False, debug=False,
                   num_devices=NCORES)

    xT_d = nc.dram_tensor("xT", [128, 32 * M_TOK], F16, kind="ExternalInput")
    csT_d = nc.dram_tensor("csT", [128, 32], F32, kind="ExternalInput")
    qwT_d = nc.dram_tensor("qwT", [IN_F // 2, NSH], U8, kind="ExternalInput")
    srep_d = nc.dram_tensor("srep", [NPAIR * 128, NSH], F16, kind="ExternalInput")
    zrT_d = nc.dram_tensor("zerosT", [32, NSH], F32, kind="ExternalInput")
    bias_d = nc.dram_tensor("bias", [1, NSH], F32, kind="ExternalInput")
    gpat_d = nc.dram_tensor("gpat", [128, NPAIR * 32], F16, kind="ExternalInput")
    y_d = nc.dram_tensor("y", [M_TOK, NSH], F32, kind="ExternalOutput")

    A = mybir.AluOpType

    with tile.TileContext(nc) as tc:
        with (
            tc.tile_pool(name="const", bufs=1) as cpool,
            tc.tile_pool(name="x16", bufs=1) as x16pool,
            tc.tile_pool(name="qb", bufs=3) as qbpool,
            tc.tile_pool(name="srep", bufs=3) as sreppool,
            tc.tile_pool(name="nib", bufs=3) as nibpool,
            tc.tile_pool(name="w", bufs=6) as wpool,
            tc.tile_pool(name="yout", bufs=2) as ypool,
            tc.tile_pool(name="ps", bufs=1, space="PSUM") as pspool,
        ):
            # ---- constants / small tensors ----
            csT = cpool.tile([128, 32], F32)
            nc.scalar.dma_start(csT[:], csT_d[:])
            rcs = cpool.tile([128, 32], F32)
            nc.vector.reciprocal(rcs[:], csT[:])

            zrT32 = cpool.tile([32, NSH], F32)
            nc.scalar.dma_start(zrT32[:], zrT_d[:])
            zT16 = cpool.tile([33, NSH], F16)
            nc.scalar.copy(zT16[:32, :], zrT32[:])
            b32 = cpool.tile([1, NSH], F32)
            nc.scalar.dma_start(b32[:], bias_d[:])
            nc.scalar.copy(zT16[32:33, :], b32[:])

            gpat = cpool.tile([128, NPAIR * 32], F16)
            nc.scalar.dma_start(gpat[:], gpat_d[:])

            # ---- x: 8 chunk DMAs (f16) into separate tiles, ACT converts ----
            w4 = 4 * M_TOK
            xraw = []
            for c in range(8):
                xr = x16pool.tile([128, w4], F16, tag=f"xraw_{c}",
                                  name=f"xraw_{c}")
                nc.scalar.dma_start(xr[:], xT_d[:, c * w4:(c + 1) * w4])
                xraw.append(xr)
            x16 = []
            for t in range(32):
                xt = x16pool.tile([128, M_TOK], F16, tag=f"x16_{t}",
                                  name=f"x16_{t}")
                src = xraw[t // 4][:, (t % 4) * M_TOK:(t % 4 + 1) * M_TOK]
                nc.scalar.mul(xt[:], src, rcs[:, t:t + 1])
                x16.append(xt)

            # ---- psum accumulators ----
            y_ps = [[pspool.tile([128, w], F32, tag=f"yps_{m}_{ci}",
                                 name=f"yps_{m}_{ci}")
                     for ci, (_, w) in enumerate(CHUNKS)] for m in range(2)]
            S_ps = pspool.tile([32, M_TOK], F32, tag="S_ps")

            # ---- main loop: software-pipelined (dequant LOOKAHEAD SPs ahead)
            wtiles = {}   # (pair, half) -> w tile

            def emit_dequant(sp):
                # qbcat: [pairA bytes | pairB bytes]
                qbcat = qbpool.tile([128, 2 * NSH], U8, tag="qb",
                                    name=f"qb_{sp}")
                nc.sync.dma_start(
                    qbcat[:].rearrange("p (j n) -> p j n", j=2),
                    qwT_d[sp * 256:(sp + 1) * 256, :]
                    .rearrange("(j p) n -> p j n", p=128))

                srepc = sreppool.tile([128, 2 * NSH], F16, tag="srep",
                                      name=f"srep_{sp}")
                nc.sync.dma_start(
                    srepc[:].rearrange("p (j n) -> p j n", j=2),
                    srep_d[sp * 256:(sp + 1) * 256, :]
                    .rearrange("(j p) n -> p j n", p=128))

                # nibcat: [loA | loB | hiA | hiB]
                nibcat = nibpool.tile([128, 4 * NSH], U8, tag="nib",
                                      name=f"nib_{sp}")
                nc.vector.tensor_scalar(
                    nibcat[:, :2 * NSH].bitcast(U16), in0=qbcat[:].bitcast(U16),
                    scalar1=0x0F0F, scalar2=None, op0=A.bitwise_and)
                nc.vector.tensor_scalar(
                    nibcat[:, 2 * NSH:].bitcast(U16), in0=qbcat[:].bitcast(U16),
                    scalar1=4, scalar2=0x0F0F,
                    op0=A.logical_shift_right, op1=A.bitwise_and)

                # per-(pair, half) mults: w = nib * srep
                for j in range(2):
                    b = 2 * sp + j
                    for half in range(2):
                        blk = (2 * half + j) * NSH
                        w = wpool.tile([128, NSH], F16, tag=f"w_{half}",
                                       name=f"w_{b}_{half}")
                        eng = nc.gpsimd if (b, half) in POOL_TT else nc.vector
                        eng.tensor_tensor(w[:], nibcat[:, blk:blk + NSH],
                                          srepc[:, j * NSH:(j + 1) * NSH],
                                          A.mult)
                        wtiles[(b, half)] = w

            def emit_mms(sp):
                for j in range(2):
                    b = 2 * sp + j
                    nc.tensor.matmul(S_ps[:], gpat[:, b * 32:(b + 1) * 32],
                                     x16[2 * b][:], start=(b == 0), stop=False)
                    nc.tensor.matmul(S_ps[:], gpat[:, b * 32:(b + 1) * 32],
                                     x16[2 * b + 1][:], start=False,
                                     stop=(b == NPAIR - 1))
                    for half, xt in ((0, x16[2 * b]), (1, x16[2 * b + 1])):
                        w = wtiles.pop((b, half))
                        for m in range(2):
                            for ci, (c0, cw) in enumerate(CHUNKS):
                                nc.tensor.matmul(
                                    y_ps[m][ci][:],
                                    xt[:, m * 128:(m + 1) * 128],
                                    w[:, c0:c0 + cw],
                                    start=(b == 0 and half == 0),
                                    stop=False,
                                )

            for sp in range(NSP + LOOKAHEAD):
                if sp < NSP:
                    emit_dequant(sp)
                if sp >= LOOKAHEAD:
                    emit_mms(sp - LOOKAHEAD)

            # ---- tail: zeros+bias augmented matmul, direct-psum stores ----
            S16 = cpool.tile([33, M_TOK], F16)
            nc.scalar.copy(S16[:32, :], S_ps[:])
            nc.vector.memset(S16[32:33, :], 1.0)
            for m in range(2):
                for ci, (c0, cw) in enumerate(CHUNKS):
                    nc.tensor.matmul(y_ps[m][ci][:],
                                     S16[:, m * 128:(m + 1) * 128],
                                     zT16[:, c0:c0 + cw],
                                     start=False, stop=True)
                    ysb = ypool.tile([128, cw], F32, tag=f"ysb_{ci}",
                                     name=f"ysb_{m}_{ci}")
                    nc.scalar.copy(ysb[:], y_ps[m][ci][:])
                    nc.scalar.dma_start(y_d[m * 128:(m + 1) * 128, c0:c0 + cw],
                                        ysb[:])
    nc.compile()
    return nc


def _host_prep(x, qweight, scales, zeros, channel_scales, bias):
    x2 = np.asarray(x, dtype=np.float32).reshape(M_TOK, IN_F)
    qw = np.asarray(qweight)
    if qw.dtype != np.uint8:
        qw = qw.astype(np.uint8)
    qwT = np.ascontiguousarray(qw.T)                      # [K/2, N]

    q = np.arange(128)
    perm = np.empty(IN_F, np.int64)
    for b in range(NPAIR):
        perm[(2 * b) * 128 + q] = 256 * b + 2 * q
        perm[(2 * b + 1) * 128 + q] = 256 * b + 2 * q + 1

    xT_perm = x2.T[perm]                                  # [K, M]
    xT_b = np.ascontiguousarray(
        xT_perm.reshape(32, 128, M_TOK).transpose(1, 0, 2)
        .reshape(128, 32 * M_TOK)).astype(np.float16)
    cs_perm = np.asarray(channel_scales, np.float32)[perm]
    csT = np.ascontiguousarray(cs_perm.reshape(32, 128).T)  # [128, 32]

    scalesT = np.asarray(scales, np.float32).T            # [32, N]
    srep = np.empty((NPAIR * 128, OUT_F), np.float16)
    for b in range(NPAIR):
        srep[b * 128:b * 128 + 64] = scalesT[2 * b].astype(np.float16)
        srep[b * 128 + 64:(b + 1) * 128] = scalesT[2 * b + 1].astype(np.float16)

    zerosT = np.ascontiguousarray(np.asarray(zeros, np.float32).T)
    bias_f = np.asarray(bias, np.float32)

    gpat = np.zeros((128, NPAIR * 32), np.float16)
    for b in range(NPAIR):
        gpat[0:64, b * 32 + 2 * b] = 1.0
        gpat[64:128, b * 32 + 2 * b + 1] = 1.0

    return xT_b, csT, qwT, srep, zerosT, bias_f, gpat


def make_in_maps(x, qweight, scales, zeros, channel_scales, bias):
    xT_b, csT, qwT, srep, zerosT, bias_f, gpat = _host_prep(
        x, qweight, scales, zeros, channel_scales, bias)
    in_maps = []
    for c in range(NCORES):
        sl = slice(c * NSH, (c + 1) * NSH)
        in_maps.append({
            "xT": xT_b,
            "csT": csT,
            "qwT": np.ascontiguousarray(qwT[:, sl]),
            "srep": np.ascontiguousarray(srep[:, sl]),
            "zerosT": np.ascontiguousarray(zerosT[:, sl]),
            "bias": np.ascontiguousarray(bias_f[sl]).reshape(1, NSH),
            "gpat": gpat,
        })
    return in_maps


_NC_CACHE = {}


def get_nc():
    if "nc" not in _NC_CACHE:
        _NC_CACHE["nc"] = _build_nc()
    return _NC_CACHE["nc"]


def kernel(x, qweight, scales, zeros, channel_scales, bias):
    in_maps = make_in_maps(x, qweight, scales, zeros, channel_scales, bias)
    nc = get_nc()
    res = run_bass_kernel_spmd(nc, in_maps, core_ids=list(range(NCORES)))
    y = np.concatenate([res.results[c]["y"] for c in range(NCORES)], axis=1)
    return y.reshape(1, M_TOK, OUT_F).astype(np.float32)


# revision 16
# speedup vs baseline: 1.2722x; 1.0223x over previous
"""AWQ W4A16-style quantized linear (nn_AWQLinear) on 8 Trainium2 NeuronCores.

y[m,n] = sum_k x[m,k] * ((wq[n,k]*scales[n,g(k)] + zeros[n,g(k)]) / cs[k]) + bias[n]

Column-parallel over out_features (8 cores, N_shard = 1376/core).

Host (layout-only): qweight transposed to byte-rows [K/2, N]; x transposed
(fp16) with a k-permutation so each 128-row k-tile pair shares one byte-row
block; scales replicated to the per-k-row broadcast pattern (fp16 round —
identical to a device-side cast).

Device, per super-pair sp (2 byte-row blocks = 4 k-tiles), all heavy
elementwise fused to minimize per-op overhead:
  nibcat[:, :2N]  = (qbcat.u16 & 0x0f0f).u8          DVE, 16-bit 2x mode
  nibcat[:, 2N:]  = ((qbcat.u16 >> 4) & 0x0f0f).u8   DVE
  wcat = nibcat * srep(broadcast)  -> f16            one TT (DVE or GPSIMD)
  12 matmuls -> 6 psum accumulators; 0/1-pattern matmuls -> S psum
Tail: zeros+bias folded into one augmented matmul (S16 row 32 = 1, zT row 32
= bias); outputs DMA'd directly from PSUM.
"""
import numpy as np

import concourse.bacc as bacc
import concourse.mybir as mybir
from concourse import tile
from concourse.bass_utils import run_bass_kernel_spmd

IN_F = 4096          # K
OUT_F = 11008        # N
M_TOK = 256          # M
NCORES = 8
NSH = OUT_F // NCORES   # 1376
NPAIR = IN_F // 256     # 16 byte-row blocks of 128 rows (each -> 2 k-tiles)
NSP = NPAIR // 2        # 8 super-pairs
CHUNKS = [(0, 512), (512, 512), (1024, NSH - 1024)]
# (pair, half) keys whose w-mult runs on GPSIMD (rest on DVE).
# GPSIMD shares SBUF ports with DVE: concurrent Pool+DVE tensor ops degrade
# DVE ~2.5x (measured), so Pool gets nothing.
POOL_TT = set()
LOOKAHEAD = 2  # super-pairs of dequant emitted ahead of their matmuls

F32, F16, U8, U16 = mybir.dt.float32, mybir.dt.float16, mybir.dt.uint8, mybir.dt.uint16


def _build_nc():
    nc = bacc.Bacc("TRN2", target_bir_lowering=False, debug=False,
                   num_devices=NCORES)

    xT_d = nc.dram_tensor("xT", [128, 32 * M_TOK], F16, kind="ExternalInput")
    csT_d = nc.dram_tensor("csT", [128, 32], F32, kind="ExternalInput")
    qwT_d = nc.dram_tensor("qwT", [IN_F // 2, NSH], U8, kind="ExternalInput")
    srep_d = nc.dram_tensor("srep", [NPAIR * 128, NSH], F16, kind="ExternalInput")
    zrT_d = nc.dram_tensor("zerosT", [32, NSH], F32, kind="ExternalInput")
    bias_d = nc.dram_tensor("bias", [1, NSH], F32, kind="ExternalInput")
    gpat_d = nc.dram_tensor("gpat", [128, NPAIR * 32], F16, kind="ExternalInput")
    y_d = nc.dram_tensor("y", [M_TOK, NSH], F32, kind="ExternalOutput")

    A = mybir.AluOpType

    with tile.TileContext(nc) as tc:
        with (
            tc.tile_pool(name="const", bufs=1) as cpool,
            tc.tile_pool(name="x16", bufs=1) as x16pool,
            tc.tile_pool(name="qb", bufs=3) as qbpool,
            tc.tile_pool(name="srep", bufs=3) as sreppool,
            tc.tile_pool(name="nib", bufs=3) as nibpool,
            tc.tile_pool(name="w", bufs=6) as wpool,
            tc.tile_pool(name="yout", bufs=2) as ypool,
            tc.tile_pool(name="ps", bufs=1, space="PSUM") as pspool,
        ):
            # ---- constants / small tensors ----
            csT = cpool.tile([128, 32], F32)
            nc.scalar.dma_start(csT[:], csT_d[:])
            rcs = cpool.tile([128, 32], F32)
            nc.vector.reciprocal(rcs[:], csT[:])

            zrT32 = cpool.tile([32, NSH], F32)
            nc.scalar.dma_start(zrT32[:], zrT_d[:])
            zT16 = cpool.tile([33, NSH], F16)
            nc.scalar.copy(zT16[:32, :], zrT32[:])
            b32 = cpool.tile([1, NSH], F32)
            nc.scalar.dma_start(b32[:], bias_d[:])
            nc.scalar.copy(zT16[32:33, :], b32[:])

            gpat = cpool.tile([128, NPAIR * 32], F16)
            nc.scalar.dma_start(gpat[:], gpat_d[:])

            # ---- x: 8 chunk DMAs (f16) into separate tiles, ACT converts ----
            w4 = 4 * M_TOK
            xraw = []
            for c in range(8):
                xr = x16pool.tile([128, w4], F16, tag=f"xraw_{c}",
                                  name=f"xraw_{c}")
                nc.scalar.dma_start(xr[:], xT_d[:, c * w4:(c + 1) * w4])
                xraw.append(xr)
            x16 = []
            for t in range(32):
                xt = x16pool.tile([128, M_TOK], F16, tag=f"x16_{t}",
                                  name=f"x16_{t}")
                src = xraw[t // 4][:, (t % 4) * M_TOK:(t % 4 + 1) * M_TOK]
                nc.scalar.mul(xt[:], src, rcs[:, t:t + 1])
                x16.append(xt)

            # ---- psum accumulators ----
            y_ps = [[pspool.tile([128, w], F32, tag=f"yps_{m}_{ci}",
                                 name=f"yps_{m}_{ci}")
                     for ci, (_, w) in enumerate(CHUNKS)] for m in range(2)]
            S_ps = pspool.tile([32, M_TOK], F32, tag="S_ps")

            # ---- main loop: software-pipelined (dequant LOOKAHEAD SPs ahead)
            wtiles = {}   # (pair, half) -> w tile

            def emit_dequant(sp):
                # qbcat: [pairA bytes | pairB bytes]
                qbcat = qbpool.tile([128, 2 * NSH], U8, tag="qb",
                                    name=f"qb_{sp}")
                nc.sync.dma_start(
                    qbcat[:].rearrange("p (j n) -> p j n", j=2),
                    qwT_d[sp * 256:(sp + 1) * 256, :]
                    .rearrange("(j p) n -> p j n", p=128))

                srepc = sreppool.tile([128, 2 * NSH], F16, tag="srep",
                                      name=f"srep_{sp}")
                nc.sync.dma_start(
                    srepc[:].rearrange("p (j n) -> p j n", j=2),
                    srep_d[sp * 256:(sp + 1) * 256, :]
                    .rearrange("(j p) n -> p j n", p=128))

                # nibcat: [loA | loB | hiA | hiB]
                nibcat = nibpool.tile([128, 4 * NSH], U8, tag="nib",
                                      name=f"nib_{sp}")
                nc.vector.tensor_scalar(
                    nibcat[:, :2 * NSH].bitcast(U16), in0=qbcat[:].bitcast(U16),
                    scalar1=0x0F0F, scalar2=None, op0=A.bitwise_and)
                nc.vector.tensor_scalar(
                    nibcat[:, 2 * NSH:].bitcast(U16), in0=qbcat[:].bitcast(U16),
                    scalar1=4, scalar2=0x0F0F,
                    op0=A.logical_shift_right, op1=A.bitwise_and)

                # per-(pair, half) mults: w = nib * srep
                for j in range(2):
                    b = 2 * sp + j
                    for half in range(2):
                        blk = (2 * half + j) * NSH
                        w = wpool.tile([128, NSH], F16, tag=f"w_{half}",
                                       name=f"w_{b}_{half}")
                        eng = nc.gpsimd if (b, half) in POOL_TT else nc.vector
                        eng.tensor_tensor(w[:], nibcat[:, blk:blk + NSH],
                                          srepc[:, j * NSH:(j + 1) * NSH],
                                          A.mult)
                        wtiles[(b, half)] = w

            def emit_mms(sp):
                for j in range(2):
                    b = 2 * sp + j
                    nc.tensor.matmul(S_ps[:], gpat[:, b * 32:(b + 1) * 32],
                                     x16[2 * b][:], start=(b == 0), stop=False)
                    nc.tensor.matmul(S_ps[:], gpat[:, b * 32:(b + 1) * 32],
                                     x16[2 * b + 1][:], start=False,
                                     stop=(b == NPAIR - 1))
                    for half, xt in ((0, x16[2 * b]), (1, x16[2 * b + 1])):
                        w = wtiles.pop((b, half))
                        for m in range(2):
                            for ci, (c0, cw) in enumerate(CHUNKS):
                                nc.tensor.matmul(
                                    y_ps[m][ci][:],
                                    xt[:, m * 128:(m + 1) * 128],
                                    w[:, c0:c0 + cw],
                                    start=(b == 0 and half == 0),
                                    stop=False,
                                )

            for sp in range(NSP + LOOKAHEAD):
                if sp < NSP:
                    emit_dequant(sp)
                if sp >= LOOKAHEAD:
                    emit_mms(sp - LOOKAHEAD)

            # ---- tail: zeros+bias augmented matmul, direct-psum stores ----
            S16 = cpool.tile([33, M_TOK], F16)
            nc.scalar.copy(S16[:32, :], S_ps[:])
            nc.vector.memset(S16[32:33, :], 1.0)
            for m in range(2):
                for ci, (c0, cw) in enumerate(CHUNKS):
                    nc.tensor.matmul(y_ps[m][ci][:],
                                     S16[:, m * 128:(m + 1) * 128],
                                     zT16[:, c0:c0 + cw],
                                     start=False, stop=True)
                    ysb = ypool.tile([128, cw], F32, tag=f"ysb_{ci}",
                                     name=f"ysb_{m}_{ci}")
                    nc.scalar.copy(ysb[:], y_ps[m][ci][:])
                    nc.scalar.dma_start(y_d[m * 128:(m + 1) * 128, c0:c0 + cw],
                                        ysb[:])
    nc.compile()
    return nc


def _host_prep(x, qweight, scales, zeros, channel_scales, bias):
    x2 = np.asarray(x, dtype=np.float32).reshape(M_TOK, IN_F)
    qw = np.asarray(qweight)
    if qw.dtype != np.uint8:
        qw = qw.astype(np.uint8)
    qwT = np.ascontiguousarray(qw.T)                      # [K/2, N]

    q = np.arange(128)
    perm = np.empty(IN_F, np.int64)
    for b in range(NPAIR):
        perm[(2 * b) * 128 + q] = 256 * b + 2 * q
        perm[(2 * b + 1) * 128 + q] = 256 * b + 2 * q + 1

    xT_perm = x2.T[perm]                                  # [K, M]
    xT_b = np.ascontiguousarray(
        xT_perm.reshape(32, 128, M_TOK).transpose(1, 0, 2)
        .reshape(128, 32 * M_TOK)).astype(np.float16)
    cs_perm = np.asarray(channel_scales, np.float32)[perm]
    csT = np.ascontiguousarray(cs_perm.reshape(32, 128).T)  # [128, 32]

    scalesT = np.asarray(scales, np.float32).T            # [32, N]
    srep = np.empty((NPAIR * 128, OUT_F), np.float16)
    for b in range(NPAIR):
        srep[b * 128:b * 128 + 64] = scalesT[2 * b].astype(np.float16)
        srep[b * 128 + 64:(b + 1) * 128] = scalesT[2 * b + 1].astype(np.float16)

    zerosT = np.ascontiguousarray(np.asarray(zeros, np.float32).T)
    bias_f = np.asarray(bias, np.float32)

    gpat = np.zeros((128, NPAIR * 32), np.float16)
    for b in range(NPAIR):
        gpat[0:64, b * 32 + 2 * b] = 1.0
        gpat[64:128, b * 32 + 2 * b + 1] = 1.0

    return xT_b, csT, qwT, srep, zerosT, bias_f, gpat


def make_in_maps(x, qweight, scales, zeros, channel_scales, bias):
    xT_b, csT, qwT, srep, zerosT, bias_f, gpat = _host_prep(
        x, qweight, scales, zeros, channel_scales, bias)
    in_maps = []
    for c in range(NCORES):
        sl = slice(c * NSH, (c + 1) * NSH)
        in_maps.append({
            "xT": xT_b,
            "csT": csT,
            "qwT": np.ascontiguousarray(qwT[:, sl]),
            "srep": np.ascontiguousarray(srep[:, sl]),
            "zerosT": np.ascontiguousarray(zerosT[:, sl]),
            "bias": np.ascontiguousarray(bias_f[sl]).reshape(1, NSH),
            "gpat": gpat,
        })
    return in_maps


_NC_CACHE = {}


def get_nc():
    if "nc" not in _NC_CACHE:
        _NC_CACHE["nc"] = _build_nc()
    return _NC_CACHE["nc"]


def kernel(x, qweight, scales, zeros, channel_scales, bias):
    in_maps = make_in_maps(x, qweight, scales, zeros, channel_scales, bias)
    nc = get_nc()
    res = run_bass_kernel_spmd(nc, in_maps, core_ids=list(range(NCORES)))
    y = np.concatenate([res.results[c]["y"] for c in range(NCORES)], axis=1)
    return y.reshape(1, M_TOK, OUT_F).astype(np.float32)


# revision 19
# speedup vs baseline: 1.4709x; 1.1562x over previous
"""AWQ W4A16-style quantized linear (nn_AWQLinear) on 8 Trainium2 NeuronCores.

y[m,n] = sum_k x[m,k] * ((wq[n,k]*scales[n,g(k)] + zeros[n,g(k)]) / cs[k]) + bias[n]

Column-parallel over out_features (8 cores, N_shard = 1376/core).

All-f16 dequant formulation (keeps every DVE op in a 2x/4x perf mode and
moves the one unavoidable 1x-rate byte conversion to the idle ACT engine):

  qb16th = qb / 16                ACT copy u8->f16 (exact: 8-bit values)
  lo16th = qb16th mod 1.0         DVE tensor_scalar (= lo/16, exact)
  wd = qb16th * srep              DVE TT f16x f16, 2x mode
  wl = lo16th * srep              DVE TT f16x f16, 2x mode
  with x-side tiles  xA = 16*x'_even - x'_odd,  xB = x'_odd:
    y = sum xA^T wl + sum xB^T wd
  (identity: hi = qb16th - lo16th, even term = (16 x'_e) * (lo/16) * s)

x-side ops are DVE tensor_scalar 4x / TT 2x. Group sums use 0/1-pattern
matmuls with coefficients 1/16 (xA) and 17/16 (xB); zeros+bias fold into one
augmented matmul at the end. GPSIMD is left idle: it shares SBUF ports with
DVE and concurrent use degrades DVE ~2.5x (measured).
"""
import numpy as np

import concourse.bacc as bacc
import concourse.mybir as mybir
from concourse import tile
from concourse.bass_utils import run_bass_kernel_spmd

IN_F = 4096          # K
OUT_F = 11008        # N
M_TOK = 256          # M
NCORES = 8
NSH = OUT_F // NCORES   # 1376
NPAIR = IN_F // 256     # 16 byte-row blocks of 128 rows (each -> 2 k-tiles)
NSP = NPAIR // 2        # 8 super-pairs
CHUNKS = [(0, 512), (512, 512), (1024, NSH - 1024)]
LOOKAHEAD = 2  # super-pairs of dequant emitted ahead of their matmuls

F32, F16, U8, U16 = mybir.dt.float32, mybir.dt.float16, mybir.dt.uint8, mybir.dt.uint16


def _build_nc():
    nc = bacc.Bacc("TRN2", target_bir_lowering=False, debug=False,
                   num_devices=NCORES)

    xT_d = nc.dram_tensor("xT", [128, 32 * M_TOK], F16, kind="ExternalInput")
    csT_d = nc.dram_tensor("csT", [128, 32], F32, kind="ExternalInput")
    qwT_d = nc.dram_tensor("qwT", [IN_F // 2, NSH], U8, kind="ExternalInput")
    srep_d = nc.dram_tensor("srep", [NPAIR * 128, NSH], F16, kind="ExternalInput")
    zrT_d = nc.dram_tensor("zerosT", [32, NSH], F32, kind="ExternalInput")
    bias_d = nc.dram_tensor("bias", [1, NSH], F32, kind="ExternalInput")
    gpat_d = nc.dram_tensor("gpat", [128, 2 * NPAIR * 32], F16,
                            kind="ExternalInput")
    y_d = nc.dram_tensor("y", [M_TOK, NSH], F32, kind="ExternalOutput")

    A = mybir.AluOpType

    with tile.TileContext(nc) as tc:
        with (
            tc.tile_pool(name="const", bufs=1) as cpool,
            tc.tile_pool(name="xop", bufs=1) as xpool,
            tc.tile_pool(name="qb", bufs=3) as qbpool,
            tc.tile_pool(name="srep", bufs=3) as sreppool,
            tc.tile_pool(name="q16", bufs=3) as q16pool,
            tc.tile_pool(name="w", bufs=4) as wpool,
            tc.tile_pool(name="yout", bufs=2) as ypool,
            tc.tile_pool(name="ps", bufs=1, space="PSUM") as pspool,
        ):
            # ---- constants ----
            csT = cpool.tile([128, 32], F32)
            nc.scalar.dma_start(csT[:], csT_d[:])
            rcs = cpool.tile([128, 32], F32)
            nc.vector.reciprocal(rcs[:], csT[:])
            rcs16 = cpool.tile([128, 32], F32)
            nc.vector.tensor_scalar(rcs16[:], in0=rcs[:], scalar1=16.0,
                                    scalar2=None, op0=A.mult)

            zrT32 = cpool.tile([32, NSH], F32)
            nc.scalar.dma_start(zrT32[:], zrT_d[:])
            zT16 = cpool.tile([33, NSH], F16)
            nc.scalar.copy(zT16[:32, :], zrT32[:])
            b32 = cpool.tile([1, NSH], F32)
            nc.scalar.dma_start(b32[:], bias_d[:])
            nc.scalar.copy(zT16[32:33, :], b32[:])

            gpat = cpool.tile([128, 2 * NPAIR * 32], F16)
            nc.scalar.dma_start(gpat[:], gpat_d[:])

            # ---- x: 8 chunk DMAs (f16) ----
            w4 = 4 * M_TOK
            xraw = []
            for c in range(8):
                xr = xpool.tile([128, w4], F16, tag=f"xraw_{c}",
                                name=f"xraw_{c}")
                nc.scalar.dma_start(xr[:], xT_d[:, c * w4:(c + 1) * w4])
                xraw.append(xr)

            def xslice(t):
                return xraw[t // 4][:, (t % 4) * M_TOK:(t % 4 + 1) * M_TOK]

            # ---- psum accumulators ----
            y_ps = [[pspool.tile([128, w], F32, tag=f"yps_{m}_{ci}",
                                 name=f"yps_{m}_{ci}")
                     for ci, (_, w) in enumerate(CHUNKS)] for m in range(2)]
            S_ps = pspool.tile([32, M_TOK], F32, tag="S_ps")

            # ---- software-pipelined main loop ----
            state = {}   # sp -> (wl, wd, {b: (xA, xB)})
            ACT_LO = {0, 2, 4, 6}   # SPs whose lo-nibble f16 convert runs on ACT

            def emit_dequant(sp):
                qbcat = qbpool.tile([128, 2 * NSH], U8, tag="qb",
                                    name=f"qb_{sp}")
                nc.sync.dma_start(
                    qbcat[:].rearrange("p (j n) -> p j n", j=2),
                    qwT_d[sp * 256:(sp + 1) * 256, :]
                    .rearrange("(j p) n -> p j n", p=128))

                srepc = sreppool.tile([128, 2 * NSH], F16, tag="srep",
                                      name=f"srep_{sp}")
                nc.sync.dma_start(
                    srepc[:].rearrange("p (j n) -> p j n", j=2),
                    srep_d[sp * 256:(sp + 1) * 256, :]
                    .rearrange("(j p) n -> p j n", p=128))

                # byte -> f16 conversion on ACT (scale 1/16: exact values)
                q16 = q16pool.tile([128, 2 * NSH], F16, tag="q16",
                                   name=f"q16_{sp}")
                nc.scalar.mul(q16[:], qbcat[:], 0.0625)
                wd = wpool.tile([128, 2 * NSH], F16, tag="wd",
                                name=f"wd_{sp}")
                nc.vector.tensor_tensor(wd[:], q16[:], srepc[:], A.mult)

                # lo nibbles: u16 fused bitwise extract (2x mode)
                lo8 = q16pool.tile([128, 2 * NSH], U8, tag="lo8",
                                   name=f"lo8_{sp}")
                nc.vector.tensor_scalar(lo8[:].bitcast(U16),
                                        in0=qbcat[:].bitcast(U16),
                                        scalar1=0x0F0F, scalar2=None,
                                        op0=A.bitwise_and)
                wl = wpool.tile([128, 2 * NSH], F16, tag="wl",
                                name=f"wl_{sp}")
                if sp in ACT_LO:
                    lo16 = q16pool.tile([128, 2 * NSH], F16, tag="lo16",
                                        name=f"lo16_{sp}")
                    nc.scalar.copy(lo16[:], lo8[:])
                    nc.vector.tensor_tensor(wl[:], lo16[:], srepc[:], A.mult)
                else:
                    nc.vector.tensor_tensor(wl[:], lo8[:], srepc[:], A.mult)

                xab = {}
                for j in range(2):
                    b = 2 * sp + j
                    te, to = 2 * b, 2 * b + 1
                    xB = xpool.tile([128, M_TOK], F16, tag="xB",
                                    bufs=6, name=f"xB_{b}")
                    nc.vector.tensor_scalar(xB[:], in0=xslice(to),
                                            scalar1=rcs[:, to:to + 1],
                                            scalar2=None, op0=A.mult)
                    xC = xpool.tile([128, M_TOK], F16, tag="xC",
                                    bufs=3, name=f"xC_{b}")
                    nc.vector.tensor_scalar(xC[:], in0=xB[:], scalar1=0.0625,
                                            scalar2=None, op0=A.mult)
                    xA = xpool.tile([128, M_TOK], F16, tag="xA",
                                    bufs=6, name=f"xA_{b}")
                    nc.vector.scalar_tensor_tensor(
                        xA[:], in0=xslice(te), scalar=rcs[:, te:te + 1],
                        in1=xC[:], op0=A.mult, op1=A.subtract)
                    xab[b] = (xA, xB)
                state[sp] = (wl, wd, xab)

            def emit_mms(sp):
                wl, wd, xab = state.pop(sp)
                for j in range(2):
                    b = 2 * sp + j
                    xA, xB = xab[b]
                    nc.tensor.matmul(S_ps[:],
                                     gpat[:, (2 * b) * 32:(2 * b + 1) * 32],
                                     xA[:], start=(b == 0), stop=False)
                    nc.tensor.matmul(S_ps[:],
                                     gpat[:, (2 * b + 1) * 32:(2 * b + 2) * 32],
                                     xB[:], start=False,
                                     stop=(b == NPAIR - 1))
                    for w, xt in ((wl, xA), (wd, xB)):
                        for m in range(2):
                            for ci, (c0, cw) in enumerate(CHUNKS):
                                nc.tensor.matmul(
                                    y_ps[m][ci][:],
                                    xt[:, m * 128:(m + 1) * 128],
                                    w[:, j * NSH + c0:j * NSH + c0 + cw],
                                    start=(b == 0 and w is wl),
                                    stop=False,
                                )

            for sp in range(NSP + LOOKAHEAD):
                if sp < NSP:
                    emit_dequant(sp)
                if sp >= LOOKAHEAD:
                    emit_mms(sp - LOOKAHEAD)

            # ---- tail: zeros+bias augmented matmul, drain ----
            S16 = cpool.tile([33, M_TOK], F16)
            nc.scalar.copy(S16[:32, :], S_ps[:])
            nc.vector.memset(S16[32:33, :], 1.0)
            for m in range(2):
                for ci, (c0, cw) in enumerate(CHUNKS):
                    nc.tensor.matmul(y_ps[m][ci][:],
                                     S16[:, m * 128:(m + 1) * 128],
                                     zT16[:, c0:c0 + cw],
                                     start=False, stop=True)
                    ysb = ypool.tile([128, cw], F32, tag=f"ysb_{ci}",
                                     name=f"ysb_{m}_{ci}")
                    nc.scalar.copy(ysb[:], y_ps[m][ci][:])
                    nc.scalar.dma_start(y_d[m * 128:(m + 1) * 128, c0:c0 + cw],
                                        ysb[:])
    nc.compile()
    return nc


def _host_prep(x, qweight, scales, zeros, channel_scales, bias):
    x2 = np.asarray(x, dtype=np.float32).reshape(M_TOK, IN_F)
    qw = np.asarray(qweight)
    if qw.dtype != np.uint8:
        qw = qw.astype(np.uint8)
    qwT = np.ascontiguousarray(qw.T)                      # [K/2, N]

    q = np.arange(128)
    perm = np.empty(IN_F, np.int64)
    for b in range(NPAIR):
        perm[(2 * b) * 128 + q] = 256 * b + 2 * q
        perm[(2 * b + 1) * 128 + q] = 256 * b + 2 * q + 1

    xT_perm = x2.T[perm]                                  # [K, M]
    xT_b = np.ascontiguousarray(
        xT_perm.reshape(32, 128, M_TOK).transpose(1, 0, 2)
        .reshape(128, 32 * M_TOK)).astype(np.float16)
    cs_perm = np.asarray(channel_scales, np.float32)[perm]
    csT = np.ascontiguousarray(cs_perm.reshape(32, 128).T)  # [128, 32]

    scalesT = np.asarray(scales, np.float32).T            # [32, N]
    srep = np.empty((NPAIR * 128, OUT_F), np.float16)
    for b in range(NPAIR):
        srep[b * 128:b * 128 + 64] = scalesT[2 * b].astype(np.float16)
        srep[b * 128 + 64:(b + 1) * 128] = scalesT[2 * b + 1].astype(np.float16)

    zerosT = np.ascontiguousarray(np.asarray(zeros, np.float32).T)
    bias_f = np.asarray(bias, np.float32)

    # per-pair patterns: block 2b for xA (coeff 1), block 2b+1 for xB (17/16)
    gpat = np.zeros((128, 2 * NPAIR * 32), np.float16)
    for b in range(NPAIR):
        for blk, val in ((2 * b, 1.0), (2 * b + 1, 1.0625)):
            gpat[0:64, blk * 32 + 2 * b] = val
            gpat[64:128, blk * 32 + 2 * b + 1] = val

    return xT_b, csT, qwT, srep, zerosT, bias_f, gpat


def make_in_maps(x, qweight, scales, zeros, channel_scales, bias):
    xT_b, csT, qwT, srep, zerosT, bias_f, gpat = _host_prep(
        x, qweight, scales, zeros, channel_scales, bias)
    in_maps = []
    for c in range(NCORES):
        sl = slice(c * NSH, (c + 1) * NSH)
        in_maps.append({
            "xT": xT_b,
            "csT": csT,
            "qwT": np.ascontiguousarray(qwT[:, sl]),
            "srep": np.ascontiguousarray(srep[:, sl]),
            "zerosT": np.ascontiguousarray(zerosT[:, sl]),
            "bias": np.ascontiguousarray(bias_f[sl]).reshape(1, NSH),
            "gpat": gpat,
        })
    return in_maps


_NC_CACHE = {}


def get_nc():
    if "nc" not in _NC_CACHE:
        _NC_CACHE["nc"] = _build_nc()
    return _NC_CACHE["nc"]


def kernel(x, qweight, scales, zeros, channel_scales, bias):
    in_maps = make_in_maps(x, qweight, scales, zeros, channel_scales, bias)
    nc = get_nc()
    res = run_bass_kernel_spmd(nc, in_maps, core_ids=list(range(NCORES)))
    y = np.concatenate([res.results[c]["y"] for c in range(NCORES)], axis=1)
    return y.reshape(1, M_TOK, OUT_F).astype(np.float32)


# revision 23
# speedup vs baseline: 1.5304x; 1.0405x over previous
"""AWQ W4A16-style quantized linear (nn_AWQLinear) on 8 Trainium2 NeuronCores.

y[m,n] = sum_k x[m,k] * ((wq[n,k]*scales[n,g(k)] + zeros[n,g(k)]) / cs[k]) + bias[n]

Column-parallel over out_features (8 cores, N_shard = 1376/core).

All-f16 dequant formulation (keeps every DVE op in a 2x/4x perf mode and
moves the one unavoidable 1x-rate byte conversion to the idle ACT engine):

  qb16th = qb / 16                ACT copy u8->f16 (exact: 8-bit values)
  lo16th = qb16th mod 1.0         DVE tensor_scalar (= lo/16, exact)
  wd = qb16th * srep              DVE TT f16x f16, 2x mode
  wl = lo16th * srep              DVE TT f16x f16, 2x mode
  with x-side tiles  xA = 16*x'_even - x'_odd,  xB = x'_odd:
    y = sum xA^T wl + sum xB^T wd
  (identity: hi = qb16th - lo16th, even term = (16 x'_e) * (lo/16) * s)

x-side ops are DVE tensor_scalar 4x / TT 2x. Group sums use 0/1-pattern
matmuls with coefficients 1/16 (xA) and 17/16 (xB); zeros+bias fold into one
augmented matmul at the end. GPSIMD is left idle: it shares SBUF ports with
DVE and concurrent use degrades DVE ~2.5x (measured).
"""
import numpy as np

import concourse.bacc as bacc
import concourse.mybir as mybir
from concourse import tile
from concourse.bass_utils import run_bass_kernel_spmd

IN_F = 4096          # K
OUT_F = 11008        # N
M_TOK = 256          # M
NCORES = 8
NSH = OUT_F // NCORES   # 1376
NPAIR = IN_F // 256     # 16 byte-row blocks of 128 rows (each -> 2 k-tiles)
NSP = NPAIR // 2        # 8 super-pairs
CHUNKS = [(0, 512), (512, 512), (1024, NSH - 1024)]
LOOKAHEAD = 2  # super-pairs of dequant emitted ahead of their matmuls

F32, F16, U8, U16 = mybir.dt.float32, mybir.dt.float16, mybir.dt.uint8, mybir.dt.uint16


def _build_nc():
    nc = bacc.Bacc("TRN2", target_bir_lowering=False, debug=False,
                   num_devices=NCORES)

    xT_d = nc.dram_tensor("xT", [128, 32 * M_TOK], F16, kind="ExternalInput")
    csT_d = nc.dram_tensor("csT", [128, 32], F32, kind="ExternalInput")
    qwT_d = nc.dram_tensor("qwT", [IN_F // 2, NSH], U8, kind="ExternalInput")
    srep_d = nc.dram_tensor("srep", [NPAIR * 128, NSH], F16, kind="ExternalInput")
    zrT_d = nc.dram_tensor("zerosT", [32, NSH], F32, kind="ExternalInput")
    bias_d = nc.dram_tensor("bias", [1, NSH], F32, kind="ExternalInput")
    gpat_d = nc.dram_tensor("gpat", [128, 2 * NPAIR * 32], F16,
                            kind="ExternalInput")
    y_d = nc.dram_tensor("y", [M_TOK, NSH], F32, kind="ExternalOutput")

    A = mybir.AluOpType

    with tile.TileContext(nc) as tc:
        with (
            tc.tile_pool(name="const", bufs=1) as cpool,
            tc.tile_pool(name="xop", bufs=1) as xpool,
            tc.tile_pool(name="qb", bufs=3) as qbpool,
            tc.tile_pool(name="srep", bufs=3) as sreppool,
            tc.tile_pool(name="q16", bufs=2) as q16pool,
            tc.tile_pool(name="w", bufs=4) as wpool,
            tc.tile_pool(name="yout", bufs=2) as ypool,
            tc.tile_pool(name="ps", bufs=1, space="PSUM") as pspool,
        ):
            # ---- hot-path constants only (tail constants loaded later) ----
            csT = cpool.tile([128, 32], F32)
            nc.scalar.dma_start(csT[:], csT_d[:])
            rcs = cpool.tile([128, 32], F32)
            nc.vector.reciprocal(rcs[:], csT[:])

            gpat = cpool.tile([128, 2 * NPAIR * 32], F16)
            nc.scalar.dma_start(gpat[:], gpat_d[:])

            # ---- x: 8 chunk DMAs (f16), issued on the scalar queue ----
            w4 = 4 * M_TOK
            xraw = []
            for c in range(8):
                xr = xpool.tile([128, w4], F16, tag=f"xraw_{c}",
                                name=f"xraw_{c}")
                nc.scalar.dma_start(xr[:], xT_d[:, c * w4:(c + 1) * w4])
                xraw.append(xr)

            def xslice(t):
                return xraw[t // 4][:, (t % 4) * M_TOK:(t % 4 + 1) * M_TOK]

            # ---- psum accumulators ----
            y_ps = [[pspool.tile([128, w], F32, tag=f"yps_{m}_{ci}",
                                 name=f"yps_{m}_{ci}")
                     for ci, (_, w) in enumerate(CHUNKS)] for m in range(2)]
            S_ps = pspool.tile([32, M_TOK], F32, tag="S_ps")

            # ---- software-pipelined main loop ----
            state = {}   # sp -> (wl, wd, {b: (xA, xB)})
            ACT_LO = {0, 2, 4, 6}   # SPs whose lo-nibble f16 convert runs on ACT

            def emit_dequant(sp):
                qbcat = qbpool.tile([128, 2 * NSH], U8, tag="qb",
                                    name=f"qb_{sp}")
                nc.sync.dma_start(
                    qbcat[:].rearrange("p (j n) -> p j n", j=2),
                    qwT_d[sp * 256:(sp + 1) * 256, :]
                    .rearrange("(j p) n -> p j n", p=128))

                srepc = sreppool.tile([128, 2 * NSH], F16, tag="srep",
                                      name=f"srep_{sp}")
                nc.sync.dma_start(
                    srepc[:].rearrange("p (j n) -> p j n", j=2),
                    srep_d[sp * 256:(sp + 1) * 256, :]
                    .rearrange("(j p) n -> p j n", p=128))

                # byte -> f16 conversion on ACT (scale 1/16: exact values)
                q16 = q16pool.tile([128, 2 * NSH], F16, tag="q16",
                                   name=f"q16_{sp}")
                nc.scalar.mul(q16[:], qbcat[:], 0.0625)
                wd = wpool.tile([128, 2 * NSH], F16, tag="wd",
                                name=f"wd_{sp}")
                nc.vector.tensor_tensor(wd[:], q16[:], srepc[:], A.mult)

                # lo nibbles: u16 fused bitwise extract (2x mode)
                lo8 = q16pool.tile([128, 2 * NSH], U8, tag="lo8",
                                   name=f"lo8_{sp}")
                nc.vector.tensor_scalar(lo8[:].bitcast(U16),
                                        in0=qbcat[:].bitcast(U16),
                                        scalar1=0x0F0F, scalar2=None,
                                        op0=A.bitwise_and)
                wl = wpool.tile([128, 2 * NSH], F16, tag="wl",
                                name=f"wl_{sp}")
                if sp in ACT_LO:
                    lo16 = q16pool.tile([128, 2 * NSH], F16, tag="lo16",
                                        name=f"lo16_{sp}")
                    nc.scalar.copy(lo16[:], lo8[:])
                    nc.vector.tensor_tensor(wl[:], lo16[:], srepc[:], A.mult)
                else:
                    nc.vector.tensor_tensor(wl[:], lo8[:], srepc[:], A.mult)

                xab = {}
                for j in range(2):
                    b = 2 * sp + j
                    te, to = 2 * b, 2 * b + 1
                    xB = xpool.tile([128, M_TOK], F16, tag="xB",
                                    bufs=6, name=f"xB_{b}")
                    nc.vector.tensor_scalar(xB[:], in0=xslice(to),
                                            scalar1=rcs[:, to:to + 1],
                                            scalar2=None, op0=A.mult)
                    xC = xpool.tile([128, M_TOK], F16, tag="xC",
                                    bufs=3, name=f"xC_{b}")
                    nc.vector.tensor_scalar(xC[:], in0=xB[:], scalar1=0.0625,
                                            scalar2=None, op0=A.mult)
                    xA = xpool.tile([128, M_TOK], F16, tag="xA",
                                    bufs=6, name=f"xA_{b}")
                    nc.vector.scalar_tensor_tensor(
                        xA[:], in0=xslice(te), scalar=rcs[:, te:te + 1],
                        in1=xC[:], op0=A.mult, op1=A.subtract)
                    xab[b] = (xA, xB)
                state[sp] = (wl, wd, xab)

            def emit_mms(sp):
                wl, wd, xab = state.pop(sp)
                for j in range(2):
                    b = 2 * sp + j
                    xA, xB = xab[b]
                    nc.tensor.matmul(S_ps[:],
                                     gpat[:, (2 * b) * 32:(2 * b + 1) * 32],
                                     xA[:], start=(b == 0), stop=False)
                    nc.tensor.matmul(S_ps[:],
                                     gpat[:, (2 * b + 1) * 32:(2 * b + 2) * 32],
                                     xB[:], start=False,
                                     stop=(b == NPAIR - 1))
                    for w, xt in ((wl, xA), (wd, xB)):
                        for m in range(2):
                            for ci, (c0, cw) in enumerate(CHUNKS):
                                nc.tensor.matmul(
                                    y_ps[m][ci][:],
                                    xt[:, m * 128:(m + 1) * 128],
                                    w[:, j * NSH + c0:j * NSH + c0 + cw],
                                    start=(b == 0 and w is wl),
                                    stop=False,
                                )

            for sp in range(NSP + LOOKAHEAD):
                if sp < NSP:
                    emit_dequant(sp)
                if sp == 0:
                    # tail-only constants: emitted after the hot path kickoff
                    zrT32 = cpool.tile([32, NSH], F32)
                    nc.sync.dma_start(zrT32[:], zrT_d[:])
                    zT16 = cpool.tile([33, NSH], F16)
                    nc.scalar.copy(zT16[:32, :], zrT32[:])
                    b32 = cpool.tile([1, NSH], F32)
                    nc.sync.dma_start(b32[:], bias_d[:])
                    nc.scalar.copy(zT16[32:33, :], b32[:])
                if sp >= LOOKAHEAD:
                    emit_mms(sp - LOOKAHEAD)

            # ---- tail: zeros+bias augmented matmul, drain ----
            S16 = cpool.tile([33, M_TOK], F16)
            nc.scalar.copy(S16[:32, :], S_ps[:])
            nc.vector.memset(S16[32:33, :], 1.0)
            for m in range(2):
                for ci, (c0, cw) in enumerate(CHUNKS):
                    nc.tensor.matmul(y_ps[m][ci][:],
                                     S16[:, m * 128:(m + 1) * 128],
                                     zT16[:, c0:c0 + cw],
                                     start=False, stop=True)
                    ysb = ypool.tile([128, cw], F32, tag=f"ysb_{ci}",
                                     name=f"ysb_{m}_{ci}")
                    nc.scalar.copy(ysb[:], y_ps[m][ci][:])
                    nc.sync.dma_start(y_d[m * 128:(m + 1) * 128, c0:c0 + cw],
                                      ysb[:])
    nc.compile()
    return nc


def _host_prep(x, qweight, scales, zeros, channel_scales, bias):
    x2 = np.asarray(x, dtype=np.float32).reshape(M_TOK, IN_F)
    qw = np.asarray(qweight)
    if qw.dtype != np.uint8:
        qw = qw.astype(np.uint8)
    qwT = np.ascontiguousarray(qw.T)                      # [K/2, N]

    q = np.arange(128)
    perm = np.empty(IN_F, np.int64)
    for b in range(NPAIR):
        perm[(2 * b) * 128 + q] = 256 * b + 2 * q
        perm[(2 * b + 1) * 128 + q] = 256 * b + 2 * q + 1

    xT_perm = x2.T[perm]                                  # [K, M]
    xT_b = np.ascontiguousarray(
        xT_perm.reshape(32, 128, M_TOK).transpose(1, 0, 2)
        .reshape(128, 32 * M_TOK)).astype(np.float16)
    cs_perm = np.asarray(channel_scales, np.float32)[perm]
    csT = np.ascontiguousarray(cs_perm.reshape(32, 128).T)  # [128, 32]

    scalesT = np.asarray(scales, np.float32).T            # [32, N]
    srep = np.empty((NPAIR * 128, OUT_F), np.float16)
    for b in range(NPAIR):
        srep[b * 128:b * 128 + 64] = scalesT[2 * b].astype(np.float16)
        srep[b * 128 + 64:(b + 1) * 128] = scalesT[2 * b + 1].astype(np.float16)

    zerosT = np.ascontiguousarray(np.asarray(zeros, np.float32).T)
    bias_f = np.asarray(bias, np.float32)

    # per-pair patterns: block 2b for xA (coeff 1), block 2b+1 for xB (17/16)
    gpat = np.zeros((128, 2 * NPAIR * 32), np.float16)
    for b in range(NPAIR):
        for blk, val in ((2 * b, 1.0), (2 * b + 1, 1.0625)):
            gpat[0:64, blk * 32 + 2 * b] = val
            gpat[64:128, blk * 32 + 2 * b + 1] = val

    return xT_b, csT, qwT, srep, zerosT, bias_f, gpat


def make_in_maps(x, qweight, scales, zeros, channel_scales, bias):
    xT_b, csT, qwT, srep, zerosT, bias_f, gpat = _host_prep(
        x, qweight, scales, zeros, channel_scales, bias)
    in_maps = []
    for c in range(NCORES):
        sl = slice(c * NSH, (c + 1) * NSH)
        in_maps.append({
            "xT": xT_b,
            "csT": csT,
            "qwT": np.ascontiguousarray(qwT[:, sl]),
            "srep": np.ascontiguousarray(srep[:, sl]),
            "zerosT": np.ascontiguousarray(zerosT[:, sl]),
            "bias": np.ascontiguousarray(bias_f[sl]).reshape(1, NSH),
            "gpat": gpat,
        })
    return in_maps


_NC_CACHE = {}


def get_nc():
    if "nc" not in _NC_CACHE:
        _NC_CACHE["nc"] = _build_nc()
    return _NC_CACHE["nc"]


def kernel(x, qweight, scales, zeros, channel_scales, bias):
    in_maps = make_in_maps(x, qweight, scales, zeros, channel_scales, bias)
    nc = get_nc()
    res = run_bass_kernel_spmd(nc, in_maps, core_ids=list(range(NCORES)))
    y = np.concatenate([res.results[c]["y"] for c in range(NCORES)], axis=1)
    return y.reshape(1, M_TOK, OUT_F).astype(np.float32)
